# revision 39
# baseline (speedup 1.0000x reference)
"""Distributed Bass kernel for nn_Attention_75514114998541.

GQA attention block (16 Q heads / 4 KV heads, head_dim 128, hidden 2048,
B=2, S=2048) with per-head RMSNorm on q/k, causal softmax, output proj.

Sharding: 8 cores = 2 (batch) x 4 (head groups). Core 4*b+g handles batch b
and heads [4g, 4g+4) (= kv head g). Wq/Wk/Wv column-sharded, Wo row-sharded;
each core emits a partial [S, HID] output (bf16), host sums partials in f32.

All matmul operands are bf16 (full PE rate at any N, half the DMA bytes);
accumulation stays f32 in PSUM. Feature-on-partition layout throughout:
  xT[hid, tok] -> QT/KT[d, tok] -> ST[k, q] -> PT -> OT[d, q] -> out[tok, hid]

Host packs xt/wq/wk/wv so every DMA moves >=2KB per partition row (1KB rows
halve DMA efficiency). Schedule: K/V projections for all strips first (xt
streamed token-hi then token-lo halves), then q-strips in REVERSE (3,2,1,0)
so the longest causal softmax chain overlaps the input-DMA prologue. Q
projections for strip s-1 are split: matmuls+Square (ACT-table-safe)
interleave into attn(s)'s per-head slots; the Sqrt/recip/bcast/STT tail runs
at the strip boundary so Sqrt never lands between Exps (2 ACT table loads
per strip). Out-proj of strip s is deferred into strip s-1's attention.

ST/exp/acc run on paired [128,1024] PSUM tiles (2 banks) to amortize ACT/DVE
per-instruction overhead. Causal masking: diag tiles get a gpsimd memset of
the dead prefix + 128x128 lower-tri mask multiply (gpsimd).
PSUM: 2x ST-pair (4) + PV accum (1) + aux raws/rows/dens/chunks (3) = 8.
"""
import contextlib
import ctypes
import os
import sys
import types

import numpy as np

sys.path.insert(0, "/opt/trn_rl_repo")

import concourse.bacc as bacc
import concourse.mybir as mybir
import concourse.tile as tile
from concourse.bass_utils import run_bass_kernel_spmd

F32 = mybir.dt.float32
BF16 = mybir.dt.bfloat16

NCORES = 8
S = 2048            # sequence length (= tokens per batch)
HID = 2048          # hidden dim
D = 128             # head dim
HQ = 4              # q heads per core
STRIP = 512         # token strip (matmul moving free dim)
NSTRIP = S // STRIP          # 4
KT = HID // 128              # 16 hidden k-tiles
EPS = 1e-6
TRACE = os.environ.get("BASS_KERNEL_TRACE", "0") == "1"


def _install_profile_shim():
    """antenv.axon_hooks shim so trace=True captures NTFF under axon."""
    if "antenv.axon_hooks" in sys.modules:
        return
    so_path = "/opt/axon/libaxon_pjrt.so"
    try:
        lib = ctypes.CDLL(so_path)
    except OSError:
        return
    if not hasattr(lib, "axon_start_nrt_profile"):
        return
    lib.axon_start_nrt_profile.argtypes = [ctypes.POINTER(ctypes.c_int64), ctypes.c_size_t]
    lib.axon_start_nrt_profile.restype = ctypes.c_int64
    lib.axon_stop_nrt_profile.argtypes = [ctypes.c_char_p]
    lib.axon_stop_nrt_profile.restype = ctypes.c_int64

    @contextlib.contextmanager
    def _hook(output_dir, device_ids):
        import jax

        jax.devices()
        if device_ids:
            ids = (ctypes.c_int64 * len(device_ids))(*device_ids)
            rc = lib.axon_start_nrt_profile(ids, len(device_ids))
        else:
            rc = lib.axon_start_nrt_profile(None, 0)
        if rc != 0:
            raise RuntimeError(f"axon_start_nrt_profile rc={rc}")
        try:
            yield
        finally:
            n = lib.axon_stop_nrt_profile(str(output_dir).encode())
            if n < 0:
                raise RuntimeError(f"axon_stop_nrt_profile rc={n}")

    mod = types.ModuleType("antenv.axon_hooks")
    state = {"hook": _hook}
    mod.set_axon_ntff_profile_hook = lambda h: state.update(hook=h)
    mod.get_axon_ntff_profile_hook = lambda: state["hook"]
    sys.modules["antenv.axon_hooks"] = mod
    try:
        import antenv

        antenv.axon_hooks = mod
    except ImportError:
        pass


def build():
    nc = bacc.Bacc("TRN2", target_bir_lowering=False, debug=False, num_devices=NCORES)

    # host-packed layouts: k-tiles concatenated along the free dim so DMA
    # rows are 2-16KB per partition
    xt_ext = nc.dram_tensor("xt", [128, KT * S], BF16, kind="ExternalInput")
    wq_ext = nc.dram_tensor("wq", [128, KT * HQ * D], BF16, kind="ExternalInput")
    wk_ext = nc.dram_tensor("wk", [128, KT * D], BF16, kind="ExternalInput")
    wv_ext = nc.dram_tensor("wv", [128, KT * D], BF16, kind="ExternalInput")
    wo_ext = nc.dram_tensor("wo", [HQ * D, HID], BF16, kind="ExternalInput")
    gq_ext = nc.dram_tensor("gq", [D, 1], F32, kind="ExternalInput")
    gk_ext = nc.dram_tensor("gk", [D, 1], F32, kind="ExternalInput")
    tri_ext = nc.dram_tensor("tri", [128, 128], BF16, kind="ExternalInput")
    ones_ext = nc.dram_tensor("ones", [128, 1], BF16, kind="ExternalInput")
    ident_ext = nc.dram_tensor("ident", [128, 128], BF16, kind="ExternalInput")
    out_ext = nc.dram_tensor("out", [S, HID], BF16, kind="ExternalOutput")

    Exp = mybir.ActivationFunctionType.Exp
    Sqrt = mybir.ActivationFunctionType.Sqrt
    Square = mybir.ActivationFunctionType.Square
    mult = mybir.AluOpType.mult
    scale_qk = float(D) ** -0.5

    with tile.TileContext(nc) as tc, contextlib.ExitStack() as ctx:
        # ---- SBUF pools
        wpool = ctx.enter_context(tc.tile_pool(name="w", bufs=1))
        cpool = ctx.enter_context(tc.tile_pool(name="c", bufs=1))
        kvp = ctx.enter_context(tc.tile_pool(name="kv", bufs=1))
        qtp = ctx.enter_context(tc.tile_pool(name="qt", bufs=5))
        qrp = ctx.enter_context(tc.tile_pool(name="qr", bufs=5))
        otp = ctx.enter_context(tc.tile_pool(name="ot", bufs=8))
        ptp = ctx.enter_context(tc.tile_pool(name="pt", bufs=4))
        scr = ctx.enter_context(tc.tile_pool(name="scr", bufs=2))
        rowp = ctx.enter_context(tc.tile_pool(name="rows", bufs=2))
        rsbp = ctx.enter_context(tc.tile_pool(name="rsb", bufs=2))
        bcp = ctx.enter_context(tc.tile_pool(name="bc", bufs=4))
        outp = ctx.enter_context(tc.tile_pool(name="outev", bufs=2))
        accp = ctx.enter_context(tc.tile_pool(name="accp", bufs=5))
        # ---- PSUM pools: 4 + 1 + 3 = 8 banks
        stps = ctx.enter_context(tc.tile_pool(name="stps", bufs=2, space="PSUM"))    # [128,1024] ST pairs + vtp
        otps = ctx.enter_context(tc.tile_pool(name="otps", bufs=1, space="PSUM"))    # [128,512] PV accum
        auxps = ctx.enter_context(tc.tile_pool(name="auxps", bufs=3, space="PSUM"))  # raws/rows/dens/chunks

        # ---- DMA program order = arrival order.
        gq_sb = cpool.tile([D, 1], F32, name="gq_sb", tag="gq_sb")
        nc.sync.dma_start(out=gq_sb[:], in_=gq_ext[:])
        gk_sb = cpool.tile([D, 1], F32, name="gk_sb", tag="gk_sb")
        nc.sync.dma_start(out=gk_sb[:], in_=gk_ext[:])
        tri_sb = cpool.tile([128, 128], BF16, name="tri_sb", tag="tri_sb")
        nc.sync.dma_start(out=tri_sb[:], in_=tri_ext[:])
        ones_sb = cpool.tile([128, 1], BF16, name="ones_sb", tag="ones_sb")
        nc.sync.dma_start(out=ones_sb[:], in_=ones_ext[:])
        ident_sb = cpool.tile([128, 128], BF16, name="ident_sb", tag="ident_sb")
        nc.sync.dma_start(out=ident_sb[:], in_=ident_ext[:])
        eps_sb = cpool.tile([1, 1], F32, name="eps_sb", tag="eps_sb")
        nc.vector.memset(eps_sb[:], EPS)

        wk_sb = wpool.tile([128, KT * D], BF16, name="wk_sb", tag="wk")
        nc.sync.dma_start(out=wk_sb[:], in_=wk_ext[:])
        xt_sb = wpool.tile([128, KT * S], BF16, name="xt_sb", tag="xt")
        # first hi-chunks early so KV3's first matmuls start ASAP, then wv,
        # then the rest of token-hi (strips 3,2), then wq, then token-lo
        for k in range(2):
            nc.sync.dma_start(out=xt_sb[:, k * S + S // 2:(k + 1) * S],
                              in_=xt_ext[:, k * S + S // 2:(k + 1) * S])
        wv_sb = wpool.tile([128, KT * D], BF16, name="wv_sb", tag="wv")
        nc.sync.dma_start(out=wv_sb[:], in_=wv_ext[:])
        for k in range(2, KT):
            nc.sync.dma_start(out=xt_sb[:, k * S + S // 2:(k + 1) * S],
                              in_=xt_ext[:, k * S + S // 2:(k + 1) * S])
        wq_sb = wpool.tile([128, KT * HQ * D], BF16, name="wq_sb", tag="wq")
        for j in range(4):
            w = KT * HQ * D // 4
            nc.sync.dma_start(out=wq_sb[:, j * w:(j + 1) * w],
                              in_=wq_ext[:, j * w:(j + 1) * w])
        for k in range(KT):
            nc.sync.dma_start(out=xt_sb[:, k * S:k * S + S // 2],
                              in_=xt_ext[:, k * S:k * S + S // 2])
        wo_t = []
        for h in range(HQ):
            wo_h = wpool.tile([128, HID], BF16, name=f"wo{h}", tag=f"wo{h}")
            nc.sync.dma_start(out=wo_h[:], in_=wo_ext[h * 128:(h + 1) * 128, :])
            wo_t.append(wo_h)

        def xt_ap(s, k):
            return xt_sb[:, k * S + s * STRIP:k * S + (s + 1) * STRIP]

        def wq_ap(k, h):
            return wq_sb[:, k * HQ * D + h * D:k * HQ * D + (h + 1) * D]

        kt_strips = [None] * NSTRIP   # K-hat-T strips [128 d, STRIP tok] bf16
        v_strips = [None] * NSTRIP    # V strips [128 tok-blk, 4*128 d] bf16
        pending_out = []              # deferred out-proj: (strip, [4 x ot_sb])

        def row_invrms_bcast(row_sb, suffix, scale):
            """row_sb [1,512] sumsq (SBUF or PSUM) -> [128,512] f32 bcast of 1/rms."""
            rms = rowp.tile([1, STRIP], F32, name=f"rms_{suffix}", tag="rows")
            nc.scalar.activation(rms[:], row_sb[:], Sqrt,
                                 bias=eps_sb[:], scale=scale)
            rr = rowp.tile([1, STRIP], F32, name=f"rr_{suffix}", tag="rrows")
            nc.vector.reciprocal_approx_fast(rr[:], rms[:])
            bc = bcp.tile([128, STRIP], F32, name=f"bc_{suffix}", tag="bc")
            nc.gpsimd.partition_broadcast(bc[:], rr[:])
            return bc

        def project_q_inline(s):
            """Full Q projection + rmsnorm for strip s (prologue only)."""
            qt_h = []
            for h in range(HQ):
                raw = auxps.tile([128, STRIP], F32, name=f"qraw{s}_{h}", tag="aux")
                for k in range(KT):
                    nc.tensor.matmul(raw[:], wq_ap(k, h), xt_ap(s, k),
                                     start=(k == 0), stop=(k == KT - 1))
                sq = scr.tile([128, STRIP], BF16, name=f"sq{s}_{h}", tag="sq")
                nc.scalar.activation(sq[:], raw[:], Square)
                row_ps = auxps.tile([1, STRIP], F32, name=f"qrow{s}_{h}", tag="aux")
                nc.tensor.matmul(row_ps[:], ones_sb[:], sq[:], start=True, stop=True)
                bc = row_invrms_bcast(row_ps, f"q{s}_{h}", 1.0 / D)
                qn = qtp.tile([128, STRIP], BF16, name=f"qt{s}_{h}", tag="qt")
                nc.vector.scalar_tensor_tensor(qn[:], raw[:], gq_sb[:], bc[:],
                                               mult, mult)
                qt_h.append(qn)
            return qt_h

        def q_raw_step(s, h, rows4):
            """Q matmuls + Square + rowsum for (s,h); norm tail deferred.

            Square is in the same ACT table set as Exp, so this is safe to
            interleave into an attention phase. The sumsq row lands in slice
            h of rows4 so ONE batched Sqrt per strip handles all heads (the
            Sqrt table-set swap happens once, after h3's row is ready).
            """
            raw = auxps.tile([128, STRIP], F32, name=f"qraw{s}_{h}", tag="aux")
            for k in range(KT):
                nc.tensor.matmul(raw[:], wq_ap(k, h), xt_ap(s, k),
                                 start=(k == 0), stop=(k == KT - 1))
            sq = scr.tile([128, STRIP], BF16, name=f"sq{s}_{h}", tag="sq")
            nc.scalar.activation(sq[:], raw[:], Square)
            qraw_sb = qrp.tile([128, STRIP], BF16, name=f"qrsb{s}_{h}", tag="qr")
            nc.vector.tensor_copy(qraw_sb[:], raw[:])
            row_ps = auxps.tile([1, STRIP], F32, name=f"qrow{s}_{h}", tag="aux")
            nc.tensor.matmul(row_ps[:], ones_sb[:], sq[:], start=True, stop=True)
            nc.vector.tensor_copy(rows4[0:1, h * STRIP:(h + 1) * STRIP], row_ps[:])
            return qraw_sb

        def finalize_q(s, raws, rows4):
            """Deferred Q norm tail: one batched Sqrt, then per-head
            recip/bcast/STT (prompt chains for attn(s) head 0)."""
            rms4 = rsbp.tile([1, HQ * STRIP], F32, name=f"rms4_{s}", tag="rms4", bufs=1)
            nc.scalar.activation(rms4[:], rows4[:], Sqrt,
                                 bias=eps_sb[:], scale=1.0 / D)
            qt_h = []
            for h, qraw_sb in enumerate(raws):
                rr = rowp.tile([1, STRIP], F32, name=f"rr_q{s}_{h}", tag="rrows")
                nc.vector.reciprocal_approx_fast(
                    rr[:], rms4[0:1, h * STRIP:(h + 1) * STRIP])
                bc = bcp.tile([128, STRIP], F32, name=f"bc_q{s}_{h}", tag="bc")
                nc.gpsimd.partition_broadcast(bc[:], rr[:])
                qn = qtp.tile([128, STRIP], BF16, name=f"qt{s}_{h}", tag="qt")
                nc.vector.scalar_tensor_tensor(qn[:], qraw_sb[:], gq_sb[:], bc[:],
                                               mult, mult)
                qt_h.append(qn)
            return qt_h

        def project_kv(s):
            """K/V projection for strip s (prologue; Sqrt is pre-exp here)."""
            kraw = auxps.tile([128, STRIP], F32, name=f"kraw{s}", tag="aux")
            for k in range(KT):
                nc.tensor.matmul(kraw[:], wk_sb[:, k * D:(k + 1) * D], xt_ap(s, k),
                                 start=(k == 0), stop=(k == KT - 1))
            sqk = scr.tile([128, STRIP], BF16, name=f"sqk{s}", tag="sq")
            nc.scalar.activation(sqk[:], kraw[:], Square)
            vraw = auxps.tile([128, STRIP], F32, name=f"vraw{s}", tag="aux")
            for k in range(KT):
                nc.tensor.matmul(vraw[:], wv_sb[:, k * D:(k + 1) * D], xt_ap(s, k),
                                 start=(k == 0), stop=(k == KT - 1))
            krow_ps = auxps.tile([1, STRIP], F32, name=f"krow{s}", tag="aux")
            nc.tensor.matmul(krow_ps[:], ones_sb[:], sqk[:], start=True, stop=True)
            bck = row_invrms_bcast(krow_ps, f"k{s}", 1.0 / D)
            kn = kvp.tile([128, STRIP], BF16, name=f"kt_strip{s}", tag=f"kt{s}")
            nc.vector.scalar_tensor_tensor(kn[:], kraw[:], gk_sb[:], bck[:], mult, mult)
            kt_strips[s] = kn
            # V: evict [d, tok] to SBUF, then transpose via stps slots
            vt_sb = scr.tile([128, STRIP], BF16, name=f"vt_sb{s}", tag=f"vtsb{s}", bufs=1)
            nc.vector.tensor_copy(vt_sb[:], vraw[:])
            vs = kvp.tile([128, 4 * D], BF16, name=f"v{s}", tag=f"v{s}")
            for tb in range(4):
                tp = stps.tile([128, D], BF16, name=f"vtp{s}_{tb}", tag="st")
                nc.tensor.transpose(tp[:], vt_sb[:, tb * 128:(tb + 1) * 128],
                                    ident_sb[:])
                nc.vector.tensor_copy(vs[:, tb * D:(tb + 1) * D], tp[:])
            v_strips[s] = vs

        def v_tile(k):
            return v_strips[k // 4][:, (k % 4) * D:((k % 4) + 1) * D]

        def kt_block(k):
            return kt_strips[k // 4][:, (k % 4) * 128:((k % 4) + 1) * 128]

        def emit_out_chunk(ps_, heads_, tb, pingpong=False):
            """Emit the 4 hs-chunks of token-block tb for pending strip ps_.

            Pairs of chunks share one [128,1024] bf16 ob tile -> one 2KB-row
            DMA per pair. pingpong alternates aux/ot PSUM banks (only legal
            when no PV accumulation is live — final strip).
            """
            tok0 = ps_ * STRIP + tb * 128
            for hp in range(2):
                ob = outp.tile([128, 2 * STRIP], BF16, name=f"ob{ps_}_{tb}_{hp}", tag="ob")
                for hh in range(2):
                    hs = 2 * hp + hh
                    if pingpong and hs % 2 == 1:
                        op_ps = otps.tile([128, STRIP], F32, name=f"op{ps_}_{tb}_{hs}", tag="ot")
                    else:
                        op_ps = auxps.tile([128, STRIP], F32, name=f"op{ps_}_{tb}_{hs}", tag="aux")
                    for h in range(HQ):
                        nc.tensor.matmul(
                            op_ps[:],
                            heads_[h][:, tb * 128:(tb + 1) * 128],
                            wo_t[h][:, hs * STRIP:(hs + 1) * STRIP],
                            start=(h == 0), stop=(h == HQ - 1),
                        )
                    if hh == 0 or pingpong:
                        nc.scalar.copy(ob[:, hh * STRIP:(hh + 1) * STRIP], op_ps[:])
                    else:
                        nc.vector.tensor_copy(ob[:, STRIP:], op_ps[:])
                nc.sync.dma_start(
                    out=out_ext[tok0:tok0 + 128, hp * 2 * STRIP:(hp + 1) * 2 * STRIP],
                    in_=ob[:],
                )

        def attn_head(s, h, qt_h, slot_work=None):
            """Attention for (strip s, head h). k-pairs diag-first then descending.

            slot_work: optional callback issued mid-stream (after the 2nd
            pair) — used for out-proj chunks and next-strip Q raw steps.
            """
            ot_ps = otps.tile([128, STRIP], F32, name=f"ot{s}_{h}", tag="ot")
            acc = accp.tile([128, 2 * STRIP], BF16, name=f"acc{s}_{h}", tag="acc")

            pairs = [(4 * s + 1, 4 * s + 0, True), (4 * s + 3, 4 * s + 2, True)]
            ks = list(range(4 * s - 1, -1, -1))
            for i in range(0, len(ks), 2):
                pairs.append((ks[i], ks[i + 1], False))

            pts = {}

            def issue_st(idx):
                k0, k1, diag = pairs[idx]
                st = stps.tile([128, 2 * STRIP], F32, name=f"st{s}_{h}_{idx}", tag="st")
                for half, k in enumerate((k0, k1)):
                    off = half * STRIP
                    j = k - 4 * s
                    m0 = 128 * j if j > 0 else 0
                    nc.tensor.matmul(
                        st[:, off + m0:off + STRIP],
                        kt_block(k), qt_h[h][:, m0:],
                        start=True, stop=True,
                    )
                pt = ptp.tile([128, 2 * STRIP], BF16, name=f"pt{s}_{h}_{idx}", tag="pt")
                if diag:
                    for half, k in enumerate((k0, k1)):
                        off = half * STRIP
                        j = k - 4 * s
                        c0 = 128 * j
                        if c0 > 0:
                            nc.gpsimd.memset(pt[:, off:off + c0], 0.0)
                        nc.scalar.activation(pt[:, off + c0:off + STRIP],
                                             st[:, off + c0:off + STRIP],
                                             Exp, scale=scale_qk)
                        nc.vector.tensor_tensor(pt[:, off + c0:off + c0 + 128],
                                                pt[:, off + c0:off + c0 + 128],
                                                tri_sb[:], mult)
                else:
                    nc.scalar.activation(pt[:], st[:], Exp, scale=scale_qk)
                pts[idx] = pt

            def issue_pv(idx):
                k0, k1, diag = pairs[idx]
                first = idx == 0
                last = idx == len(pairs) - 1
                for half, k in enumerate((k0, k1)):
                    off = half * STRIP
                    j = k - 4 * s
                    m0 = 128 * j if j > 0 else 0
                    nc.tensor.matmul(
                        ot_ps[:, m0:], v_tile(k), pts[idx][:, off + m0:off + STRIP],
                        start=(first and half == 0),
                        stop=(last and half == 1),
                    )
                if first:
                    nc.vector.tensor_copy(acc[:], pts[idx][:])
                else:
                    nc.vector.tensor_add(acc[:], acc[:], pts[idx][:])
                del pts[idx]

            issue_st(0)
            for i in range(1, len(pairs)):
                issue_st(i)
                issue_pv(i - 1)
                if slot_work is not None and i == 1:
                    slot_work()
            issue_pv(len(pairs) - 1)
            if slot_work is not None and len(pairs) == 1:
                slot_work()

            # evict OT unnormalized (frees otps bank); den + normalize follow
            # per head. The last head's evict goes to ACT (idle at strip end)
            # so the next strip's first PV isn't gated on the DVE backlog.
            ot_sb = otp.tile([128, STRIP], BF16, name=f"otsb{s}_{h}", tag="ot")
            if h == HQ - 1:
                nc.scalar.copy(ot_sb[:], ot_ps[:])
            else:
                nc.vector.tensor_copy(ot_sb[:], ot_ps[:])
            return ot_sb, acc

        def finalize_strip(s, heads_acc):
            """Softmax denominators + in-place OT normalization for all 4 heads."""
            for h, (ot_sb, acc) in enumerate(heads_acc):
                den_ps = auxps.tile([1, STRIP], F32, name=f"den{s}_{h}", tag="aux")
                nc.tensor.matmul(den_ps[:], ones_sb[:], acc[:, :STRIP],
                                 start=True, stop=False)
                nc.tensor.matmul(den_ps[:], ones_sb[:], acc[:, STRIP:],
                                 start=False, stop=True)
                dr = rowp.tile([1, STRIP], F32, name=f"dr{s}_{h}", tag="rrows")
                nc.vector.reciprocal_approx_fast(dr[:], den_ps[:])
                bcd = bcp.tile([128, STRIP], F32, name=f"dbc{s}_{h}", tag="bc")
                nc.gpsimd.partition_broadcast(bcd[:], dr[:])
                nc.vector.tensor_tensor(ot_sb[:], ot_sb[:], bcd[:], mult)
            return [ot for ot, _ in heads_acc]

        # ================= schedule =================
        # Prologue (DMA-bound): KV3, KV2, Q3, KV1, KV0 — ordered to ride the
        # xt DMA stream (token-hi, wq, token-lo). All Sqrts land pre-Exp.
        project_kv(3)
        project_kv(2)
        qt3 = project_q_inline(3)
        project_kv(1)
        project_kv(0)

        RAW_PLAN = {0: (0,), 1: (1,), 2: (2,), 3: (3,)}

        # attn(3): interleave next strip's Q raw steps (Square only — table-safe)
        # den chains issue per head (early aux-ring slots) rather than batched.
        q_raws = []
        rows4 = rsbp.tile([1, HQ * STRIP], F32, name="rows4_2", tag="rows4")
        finalized = []
        for h in range(HQ):
            def work3(h=h):
                for r in RAW_PLAN[h]:
                    q_raws.append(q_raw_step(2, r, rows4))
            ha = attn_head(3, h, qt3, slot_work=work3)
            finalized.append(finalize_strip(3, [ha])[0])
        qt_next = finalize_q(2, q_raws, rows4)
        pending_out.append((3, finalized))

        # strips 2, 1: attention + pending out-proj + next strip's Q raws
        for s in (2, 1):
            qt_s = qt_next
            q_raws = []
            rows4 = rsbp.tile([1, HQ * STRIP], F32, name=f"rows4_{s-1}", tag="rows4")
            finalized = []
            for h in range(HQ):
                def work(h=h, s=s, rows4=rows4):
                    ps_, heads_ = pending_out[0]
                    emit_out_chunk(ps_, heads_, h)
                    for r in RAW_PLAN[h]:
                        q_raws.append(q_raw_step(s - 1, r, rows4))
                ha = attn_head(s, h, qt_s, slot_work=work)
                finalized.append(finalize_strip(s, [ha])[0])
            pending_out.pop(0)
            qt_next = finalize_q(s - 1, q_raws, rows4)
            pending_out.append((s, finalized))

        # strip 0: attention + pending out-proj; den chains per head so the
        # tail's final out-proj isn't gated on a batched finalize
        qt_s = qt_next
        final_heads = []
        for h in range(HQ):
            def work0(h=h):
                ps_, heads_ = pending_out[0]
                emit_out_chunk(ps_, heads_, h)
            ha = attn_head(0, h, qt_s, slot_work=work0)
            final_heads.append(finalize_strip(0, [ha])[0])
        pending_out.pop(0)

        # final out-proj for strip 0 (otps free: ping-pong banks)
        for tb in range(4):
            emit_out_chunk(0, final_heads, tb, pingpong=True)

    nc.compile()
    return nc


_NC_CACHE = None
last_result = None


def _tri_np():
    kr = np.arange(128)[:, None]
    qc = np.arange(128)[None, :]
    return np.where(kr <= qc, 1.0, 0.0).astype(np.float32)


def _pack_ktiles(a):
    """[KT*128, W] -> [128, KT*W] (k-tiles concatenated along free dim)."""
    kt, rem = divmod(a.shape[0], 128)
    assert rem == 0
    w = a.shape[1]
    return np.ascontiguousarray(
        a.reshape(kt, 128, w).transpose(1, 0, 2).reshape(128, kt * w))


def kernel(x, Wq, Wk, Wv, Wo, gq, gk):
    global _NC_CACHE, last_result
    import ml_dtypes

    bf16 = ml_dtypes.bfloat16
    x = np.asarray(x, np.float32)
    Wq = np.asarray(Wq, np.float32)
    Wk = np.asarray(Wk, np.float32)
    Wv = np.asarray(Wv, np.float32)
    Wo = np.asarray(Wo, np.float32)
    gq = np.asarray(gq, np.float32)
    gk = np.asarray(gk, np.float32)

    tri = _tri_np().astype(bf16)
    ones = np.ones((128, 1), np.float32).astype(bf16)
    ident = np.eye(128, dtype=np.float32).astype(bf16)
    in_maps = []
    for core in range(NCORES):
        b, g = core // 4, core % 4
        in_maps.append({
            "xt": _pack_ktiles(np.ascontiguousarray(x[b].T)).astype(bf16),
            "wq": _pack_ktiles(Wq[:, g * HQ * D:(g + 1) * HQ * D]).astype(bf16),
            "wk": _pack_ktiles(Wk[:, g * D:(g + 1) * D]).astype(bf16),
            "wv": _pack_ktiles(Wv[:, g * D:(g + 1) * D]).astype(bf16),
            "wo": np.ascontiguousarray(Wo[g * HQ * D:(g + 1) * HQ * D, :]).astype(bf16),
            "gq": np.ascontiguousarray(gq.reshape(D, 1)),
            "gk": np.ascontiguousarray(gk.reshape(D, 1)),
            "tri": tri,
            "ones": ones,
            "ident": ident,
        })

    if TRACE:
        _install_profile_shim()
    if _NC_CACHE is None:
        _NC_CACHE = build()
    last_result = run_bass_kernel_spmd(
        _NC_CACHE, in_maps, core_ids=list(range(NCORES)), trace=TRACE
    )
    out = np.zeros((2, S, HID), np.float32)
    for core in range(NCORES):
        out[core // 4] += np.asarray(last_result.results[core]["out"]).astype(np.float32)
    return out


# revision 41
# speedup vs baseline: 1.0104x; 1.0104x over previous
"""Distributed Bass kernel for nn_Attention_75514114998541.

GQA attention block (16 Q heads / 4 KV heads, head_dim 128, hidden 2048,
B=2, S=2048) with per-head RMSNorm on q/k, causal softmax, output proj.

Sharding: 8 cores = 2 (batch) x 4 (head groups). Core 4*b+g handles batch b
and heads [4g, 4g+4) (= kv head g). Wq/Wk/Wv column-sharded, Wo row-sharded;
each core emits a partial [S, HID] output (bf16), host sums partials in f32.

All matmul operands are bf16 (full PE rate at any N, half the DMA bytes);
accumulation stays f32 in PSUM. Feature-on-partition layout throughout:
  xT[hid, tok] -> QT/KT[d, tok] -> ST[k, q] -> PT -> OT[d, q] -> out[tok, hid]

Host packs xt/wq/wk/wv so every DMA moves >=2KB per partition row (1KB rows
halve DMA efficiency). Schedule: K/V projections for all strips first (xt
streamed token-hi then token-lo halves), then q-strips in REVERSE (3,2,1,0)
so the longest causal softmax chain overlaps the input-DMA prologue. Q
projections for strip s-1 are split: matmuls+Square (ACT-table-safe)
interleave into attn(s)'s per-head slots; the Sqrt/recip/bcast/STT tail runs
at the strip boundary so Sqrt never lands between Exps (2 ACT table loads
per strip). Out-proj of strip s is deferred into strip s-1's attention.

ST/exp/acc run on paired [128,1024] PSUM tiles (2 banks) to amortize ACT/DVE
per-instruction overhead. Causal masking: diag tiles get a gpsimd memset of
the dead prefix + 128x128 lower-tri mask multiply (gpsimd).
PSUM: 2x ST-pair (4) + PV accum (1) + aux raws/rows/dens/chunks (3) = 8.
"""
import contextlib
import ctypes
import os
import sys
import types

import numpy as np

sys.path.insert(0, "/opt/trn_rl_repo")

import concourse.bacc as bacc
import concourse.mybir as mybir
import concourse.tile as tile
from concourse.bass_utils import run_bass_kernel_spmd

F32 = mybir.dt.float32
BF16 = mybir.dt.bfloat16

NCORES = 8
S = 2048            # sequence length (= tokens per batch)
HID = 2048          # hidden dim
D = 128             # head dim
HQ = 4              # q heads per core
STRIP = 512         # token strip (matmul moving free dim)
NSTRIP = S // STRIP          # 4
KT = HID // 128              # 16 hidden k-tiles
EPS = 1e-6
TRACE = os.environ.get("BASS_KERNEL_TRACE", "0") == "1"


def _install_profile_shim():
    """antenv.axon_hooks shim so trace=True captures NTFF under axon."""
    if "antenv.axon_hooks" in sys.modules:
        return
    so_path = "/opt/axon/libaxon_pjrt.so"
    try:
        lib = ctypes.CDLL(so_path)
    except OSError:
        return
    if not hasattr(lib, "axon_start_nrt_profile"):
        return
    lib.axon_start_nrt_profile.argtypes = [ctypes.POINTER(ctypes.c_int64), ctypes.c_size_t]
    lib.axon_start_nrt_profile.restype = ctypes.c_int64
    lib.axon_stop_nrt_profile.argtypes = [ctypes.c_char_p]
    lib.axon_stop_nrt_profile.restype = ctypes.c_int64

    @contextlib.contextmanager
    def _hook(output_dir, device_ids):
        import jax

        jax.devices()
        if device_ids:
            ids = (ctypes.c_int64 * len(device_ids))(*device_ids)
            rc = lib.axon_start_nrt_profile(ids, len(device_ids))
        else:
            rc = lib.axon_start_nrt_profile(None, 0)
        if rc != 0:
            raise RuntimeError(f"axon_start_nrt_profile rc={rc}")
        try:
            yield
        finally:
            n = lib.axon_stop_nrt_profile(str(output_dir).encode())
            if n < 0:
                raise RuntimeError(f"axon_stop_nrt_profile rc={n}")

    mod = types.ModuleType("antenv.axon_hooks")
    state = {"hook": _hook}
    mod.set_axon_ntff_profile_hook = lambda h: state.update(hook=h)
    mod.get_axon_ntff_profile_hook = lambda: state["hook"]
    sys.modules["antenv.axon_hooks"] = mod
    try:
        import antenv

        antenv.axon_hooks = mod
    except ImportError:
        pass


def build():
    nc = bacc.Bacc("TRN2", target_bir_lowering=False, debug=False, num_devices=NCORES)

    # host-packed layouts: k-tiles concatenated along the free dim so DMA
    # rows are 2-16KB per partition
    xt_ext = nc.dram_tensor("xt", [128, KT * S], BF16, kind="ExternalInput")
    wq_ext = nc.dram_tensor("wq", [128, KT * HQ * D], BF16, kind="ExternalInput")
    wk_ext = nc.dram_tensor("wk", [128, KT * D], BF16, kind="ExternalInput")
    wv_ext = nc.dram_tensor("wv", [128, KT * D], BF16, kind="ExternalInput")
    wo_ext = nc.dram_tensor("wo", [HQ * D, HID], BF16, kind="ExternalInput")
    gq_ext = nc.dram_tensor("gq", [D, 1], F32, kind="ExternalInput")
    gk_ext = nc.dram_tensor("gk", [D, 1], F32, kind="ExternalInput")
    tri_ext = nc.dram_tensor("tri", [128, 128], BF16, kind="ExternalInput")
    ones_ext = nc.dram_tensor("ones", [128, 1], BF16, kind="ExternalInput")
    ident_ext = nc.dram_tensor("ident", [128, 128], BF16, kind="ExternalInput")
    out_ext = nc.dram_tensor("out", [S, HID], BF16, kind="ExternalOutput")

    Exp = mybir.ActivationFunctionType.Exp
    Sqrt = mybir.ActivationFunctionType.Sqrt
    Square = mybir.ActivationFunctionType.Square
    mult = mybir.AluOpType.mult
    scale_qk = float(D) ** -0.5

    with tile.TileContext(nc) as tc, contextlib.ExitStack() as ctx:
        # ---- SBUF pools
        wpool = ctx.enter_context(tc.tile_pool(name="w", bufs=1))
        cpool = ctx.enter_context(tc.tile_pool(name="c", bufs=1))
        kvp = ctx.enter_context(tc.tile_pool(name="kv", bufs=1))
        qtp = ctx.enter_context(tc.tile_pool(name="qt", bufs=5))
        qrp = ctx.enter_context(tc.tile_pool(name="qr", bufs=5))
        otp = ctx.enter_context(tc.tile_pool(name="ot", bufs=8))
        ptp = ctx.enter_context(tc.tile_pool(name="pt", bufs=4))
        scr = ctx.enter_context(tc.tile_pool(name="scr", bufs=2))
        rowp = ctx.enter_context(tc.tile_pool(name="rows", bufs=2))
        rsbp = ctx.enter_context(tc.tile_pool(name="rsb", bufs=2))
        bcp = ctx.enter_context(tc.tile_pool(name="bc", bufs=4))
        outp = ctx.enter_context(tc.tile_pool(name="outev", bufs=2))
        accp = ctx.enter_context(tc.tile_pool(name="accp", bufs=5))
        # ---- PSUM pools: 4 + 1 + 3 = 8 banks
        stps = ctx.enter_context(tc.tile_pool(name="stps", bufs=2, space="PSUM"))    # [128,1024] ST pairs + vtp
        otps = ctx.enter_context(tc.tile_pool(name="otps", bufs=1, space="PSUM"))    # [128,512] PV accum
        auxps = ctx.enter_context(tc.tile_pool(name="auxps", bufs=3, space="PSUM"))  # raws/rows/dens/chunks

        # ---- DMA program order = arrival order.
        gq_sb = cpool.tile([D, 1], F32, name="gq_sb", tag="gq_sb")
        nc.sync.dma_start(out=gq_sb[:], in_=gq_ext[:])
        gk_sb = cpool.tile([D, 1], F32, name="gk_sb", tag="gk_sb")
        nc.sync.dma_start(out=gk_sb[:], in_=gk_ext[:])
        tri_sb = cpool.tile([128, 128], BF16, name="tri_sb", tag="tri_sb")
        nc.sync.dma_start(out=tri_sb[:], in_=tri_ext[:])
        ones_sb = cpool.tile([128, 1], BF16, name="ones_sb", tag="ones_sb")
        nc.sync.dma_start(out=ones_sb[:], in_=ones_ext[:])
        ident_sb = cpool.tile([128, 128], BF16, name="ident_sb", tag="ident_sb")
        nc.sync.dma_start(out=ident_sb[:], in_=ident_ext[:])
        eps_sb = cpool.tile([1, 1], F32, name="eps_sb", tag="eps_sb")
        nc.vector.memset(eps_sb[:], EPS)

        wk_sb = wpool.tile([128, KT * D], BF16, name="wk_sb", tag="wk")
        nc.sync.dma_start(out=wk_sb[:], in_=wk_ext[:])
        xt_sb = wpool.tile([128, KT * S], BF16, name="xt_sb", tag="xt")
        # first hi-chunks early so KV3's first matmuls start ASAP, then wv,
        # then the rest of token-hi (strips 3,2), then wq, then token-lo
        for k in range(2):
            nc.sync.dma_start(out=xt_sb[:, k * S + S // 2:(k + 1) * S],
                              in_=xt_ext[:, k * S + S // 2:(k + 1) * S])
        wv_sb = wpool.tile([128, KT * D], BF16, name="wv_sb", tag="wv")
        nc.sync.dma_start(out=wv_sb[:], in_=wv_ext[:])
        for k in range(2, KT):
            nc.sync.dma_start(out=xt_sb[:, k * S + S // 2:(k + 1) * S],
                              in_=xt_ext[:, k * S + S // 2:(k + 1) * S])
        wq_sb = wpool.tile([128, KT * HQ * D], BF16, name="wq_sb", tag="wq")
        for j in range(4):
            w = KT * HQ * D // 4
            nc.sync.dma_start(out=wq_sb[:, j * w:(j + 1) * w],
                              in_=wq_ext[:, j * w:(j + 1) * w])
        for k in range(KT):
            nc.sync.dma_start(out=xt_sb[:, k * S:k * S + S // 2],
                              in_=xt_ext[:, k * S:k * S + S // 2])
        wo_t = []
        for h in range(HQ):
            wo_h = wpool.tile([128, HID], BF16, name=f"wo{h}", tag=f"wo{h}")
            nc.sync.dma_start(out=wo_h[:], in_=wo_ext[h * 128:(h + 1) * 128, :])
            wo_t.append(wo_h)

        def xt_ap(s, k):
            return xt_sb[:, k * S + s * STRIP:k * S + (s + 1) * STRIP]

        def wq_ap(k, h):
            return wq_sb[:, k * HQ * D + h * D:k * HQ * D + (h + 1) * D]

        kt_strips = [None] * NSTRIP   # K-hat-T strips [128 d, STRIP tok] bf16
        v_strips = [None] * NSTRIP    # V strips [128 tok-blk, 4*128 d] bf16
        pending_out = []              # deferred out-proj: (strip, [4 x ot_sb])

        def row_invrms_bcast(row_sb, suffix, scale):
            """row_sb [1,512] sumsq (SBUF or PSUM) -> [128,512] f32 bcast of 1/rms."""
            rms = rowp.tile([1, STRIP], F32, name=f"rms_{suffix}", tag="rows")
            nc.scalar.activation(rms[:], row_sb[:], Sqrt,
                                 bias=eps_sb[:], scale=scale)
            rr = rowp.tile([1, STRIP], F32, name=f"rr_{suffix}", tag="rrows")
            nc.vector.reciprocal_approx_fast(rr[:], rms[:])
            bc = bcp.tile([128, STRIP], F32, name=f"bc_{suffix}", tag="bc")
            nc.gpsimd.partition_broadcast(bc[:], rr[:])
            return bc

        def project_q_inline(s):
            """Full Q projection + rmsnorm for strip s (prologue only)."""
            qt_h = []
            for h in range(HQ):
                raw = auxps.tile([128, STRIP], F32, name=f"qraw{s}_{h}", tag="aux")
                for k in range(KT):
                    nc.tensor.matmul(raw[:], wq_ap(k, h), xt_ap(s, k),
                                     start=(k == 0), stop=(k == KT - 1))
                sq = scr.tile([128, STRIP], BF16, name=f"sq{s}_{h}", tag="sq")
                nc.scalar.activation(sq[:], raw[:], Square)
                row_ps = auxps.tile([1, STRIP], F32, name=f"qrow{s}_{h}", tag="aux")
                nc.tensor.matmul(row_ps[:], ones_sb[:], sq[:], start=True, stop=True)
                bc = row_invrms_bcast(row_ps, f"q{s}_{h}", 1.0 / D)
                qn = qtp.tile([128, STRIP], BF16, name=f"qt{s}_{h}", tag="qt")
                nc.vector.scalar_tensor_tensor(qn[:], raw[:], gq_sb[:], bc[:],
                                               mult, mult)
                qt_h.append(qn)
            return qt_h

        def q_raw_step(s, h, rows4):
            """Q matmuls + Square + rowsum for (s,h); norm tail deferred.

            Square is in the same ACT table set as Exp, so this is safe to
            interleave into an attention phase. The sumsq row lands in slice
            h of rows4 so ONE batched Sqrt per strip handles all heads (the
            Sqrt table-set swap happens once, after h3's row is ready).
            """
            raw = auxps.tile([128, STRIP], F32, name=f"qraw{s}_{h}", tag="aux")
            for k in range(KT):
                nc.tensor.matmul(raw[:], wq_ap(k, h), xt_ap(s, k),
                                 start=(k == 0), stop=(k == KT - 1))
            sq = scr.tile([128, STRIP], BF16, name=f"sq{s}_{h}", tag="sq")
            nc.scalar.activation(sq[:], raw[:], Square)
            qraw_sb = qrp.tile([128, STRIP], BF16, name=f"qrsb{s}_{h}", tag="qr")
            nc.vector.tensor_copy(qraw_sb[:], raw[:])
            row_ps = auxps.tile([1, STRIP], F32, name=f"qrow{s}_{h}", tag="aux")
            nc.tensor.matmul(row_ps[:], ones_sb[:], sq[:], start=True, stop=True)
            nc.vector.tensor_copy(rows4[0:1, h * STRIP:(h + 1) * STRIP], row_ps[:])
            return qraw_sb

        def finalize_q(s, raws, rows4):
            """Deferred Q norm tail: one batched Sqrt, then per-head
            recip/bcast/STT (prompt chains for attn(s) head 0)."""
            rms4 = rsbp.tile([1, HQ * STRIP], F32, name=f"rms4_{s}", tag="rms4", bufs=1)
            nc.scalar.activation(rms4[:], rows4[:], Sqrt,
                                 bias=eps_sb[:], scale=1.0 / D)
            qt_h = []
            for h, qraw_sb in enumerate(raws):
                rr = rowp.tile([1, STRIP], F32, name=f"rr_q{s}_{h}", tag="rrows")
                nc.vector.reciprocal_approx_fast(
                    rr[:], rms4[0:1, h * STRIP:(h + 1) * STRIP])
                bc = bcp.tile([128, STRIP], F32, name=f"bc_q{s}_{h}", tag="bc")
                nc.gpsimd.partition_broadcast(bc[:], rr[:])
                qn = qtp.tile([128, STRIP], BF16, name=f"qt{s}_{h}", tag="qt")
                nc.vector.scalar_tensor_tensor(qn[:], qraw_sb[:], gq_sb[:], bc[:],
                                               mult, mult)
                qt_h.append(qn)
            return qt_h

        def project_kv(s):
            """K/V projection for strip s (prologue; Sqrt is pre-exp here)."""
            kraw = auxps.tile([128, STRIP], F32, name=f"kraw{s}", tag="aux")
            for k in range(KT):
                nc.tensor.matmul(kraw[:], wk_sb[:, k * D:(k + 1) * D], xt_ap(s, k),
                                 start=(k == 0), stop=(k == KT - 1))
            sqk = scr.tile([128, STRIP], BF16, name=f"sqk{s}", tag="sq")
            nc.scalar.activation(sqk[:], kraw[:], Square)
            vraw = auxps.tile([128, STRIP], F32, name=f"vraw{s}", tag="aux")
            for k in range(KT):
                nc.tensor.matmul(vraw[:], wv_sb[:, k * D:(k + 1) * D], xt_ap(s, k),
                                 start=(k == 0), stop=(k == KT - 1))
            krow_ps = auxps.tile([1, STRIP], F32, name=f"krow{s}", tag="aux")
            nc.tensor.matmul(krow_ps[:], ones_sb[:], sqk[:], start=True, stop=True)
            bck = row_invrms_bcast(krow_ps, f"k{s}", 1.0 / D)
            kn = kvp.tile([128, STRIP], BF16, name=f"kt_strip{s}", tag=f"kt{s}")
            nc.vector.scalar_tensor_tensor(kn[:], kraw[:], gk_sb[:], bck[:], mult, mult)
            kt_strips[s] = kn
            # V: evict [d, tok] to SBUF, then transpose via stps slots
            vt_sb = scr.tile([128, STRIP], BF16, name=f"vt_sb{s}", tag=f"vtsb{s}", bufs=1)
            nc.vector.tensor_copy(vt_sb[:], vraw[:])
            vs = kvp.tile([128, 4 * D], BF16, name=f"v{s}", tag=f"v{s}")
            for tb in range(4):
                tp = stps.tile([128, D], BF16, name=f"vtp{s}_{tb}", tag="st")
                nc.tensor.transpose(tp[:], vt_sb[:, tb * 128:(tb + 1) * 128],
                                    ident_sb[:])
                nc.vector.tensor_copy(vs[:, tb * D:(tb + 1) * D], tp[:])
            v_strips[s] = vs

        def v_tile(k):
            return v_strips[k // 4][:, (k % 4) * D:((k % 4) + 1) * D]

        def kt_block(k):
            return kt_strips[k // 4][:, (k % 4) * 128:((k % 4) + 1) * 128]

        def emit_out_chunk(ps_, heads_, tb, pingpong=False):
            """Emit the 4 hs-chunks of token-block tb for pending strip ps_.

            Pairs of chunks share one [128,1024] bf16 ob tile -> one 2KB-row
            DMA per pair. pingpong alternates aux/ot PSUM banks (only legal
            when no PV accumulation is live — final strip).
            """
            tok0 = ps_ * STRIP + tb * 128
            for hp in range(2):
                ob = outp.tile([128, 2 * STRIP], BF16, name=f"ob{ps_}_{tb}_{hp}", tag="ob")
                for hh in range(2):
                    hs = 2 * hp + hh
                    if pingpong and hs % 2 == 1:
                        op_ps = otps.tile([128, STRIP], F32, name=f"op{ps_}_{tb}_{hs}", tag="ot")
                    else:
                        op_ps = auxps.tile([128, STRIP], F32, name=f"op{ps_}_{tb}_{hs}", tag="aux")
                    for h in range(HQ):
                        nc.tensor.matmul(
                            op_ps[:],
                            heads_[h][:, tb * 128:(tb + 1) * 128],
                            wo_t[h][:, hs * STRIP:(hs + 1) * STRIP],
                            start=(h == 0), stop=(h == HQ - 1),
                        )
                    if hh == 0 or pingpong:
                        nc.scalar.copy(ob[:, hh * STRIP:(hh + 1) * STRIP], op_ps[:])
                    else:
                        nc.vector.tensor_copy(ob[:, STRIP:], op_ps[:])
                nc.sync.dma_start(
                    out=out_ext[tok0:tok0 + 128, hp * 2 * STRIP:(hp + 1) * 2 * STRIP],
                    in_=ob[:],
                )

        def attn_head(s, h, qt_h, slot_work=None):
            """Attention for (strip s, head h). k-pairs diag-first then descending.

            slot_work: optional callback issued mid-stream (after the 2nd
            pair) — used for out-proj chunks and next-strip Q raw steps.
            """
            ot_ps = otps.tile([128, STRIP], F32, name=f"ot{s}_{h}", tag="ot")
            acc = accp.tile([128, 2 * STRIP], BF16, name=f"acc{s}_{h}", tag="acc")

            pairs = [(4 * s + 1, 4 * s + 0, True), (4 * s + 3, 4 * s + 2, True)]
            ks = list(range(4 * s - 1, -1, -1))
            for i in range(0, len(ks), 2):
                pairs.append((ks[i], ks[i + 1], False))

            pts = {}

            def issue_st(idx):
                k0, k1, diag = pairs[idx]
                st = stps.tile([128, 2 * STRIP], F32, name=f"st{s}_{h}_{idx}", tag="st")
                for half, k in enumerate((k0, k1)):
                    off = half * STRIP
                    j = k - 4 * s
                    m0 = 128 * j if j > 0 else 0
                    nc.tensor.matmul(
                        st[:, off + m0:off + STRIP],
                        kt_block(k), qt_h[h][:, m0:],
                        start=True, stop=True,
                    )
                pt = ptp.tile([128, 2 * STRIP], BF16, name=f"pt{s}_{h}_{idx}", tag="pt")
                if diag:
                    for half, k in enumerate((k0, k1)):
                        off = half * STRIP
                        j = k - 4 * s
                        c0 = 128 * j
                        if c0 > 0:
                            nc.gpsimd.memset(pt[:, off:off + c0], 0.0)
                        nc.scalar.activation(pt[:, off + c0:off + STRIP],
                                             st[:, off + c0:off + STRIP],
                                             Exp, scale=scale_qk)
                        nc.vector.tensor_tensor(pt[:, off + c0:off + c0 + 128],
                                                pt[:, off + c0:off + c0 + 128],
                                                tri_sb[:], mult)
                else:
                    nc.scalar.activation(pt[:], st[:], Exp, scale=scale_qk)
                pts[idx] = pt

            def issue_pv(idx):
                k0, k1, diag = pairs[idx]
                first = idx == 0
                last = idx == len(pairs) - 1
                for half, k in enumerate((k0, k1)):
                    off = half * STRIP
                    j = k - 4 * s
                    m0 = 128 * j if j > 0 else 0
                    nc.tensor.matmul(
                        ot_ps[:, m0:], v_tile(k), pts[idx][:, off + m0:off + STRIP],
                        start=(first and half == 0),
                        stop=(last and half == 1),
                    )
                if first:
                    nc.vector.tensor_copy(acc[:], pts[idx][:])
                else:
                    nc.vector.tensor_add(acc[:], acc[:], pts[idx][:])
                del pts[idx]

            issue_st(0)
            for i in range(1, len(pairs)):
                issue_st(i)
                issue_pv(i - 1)
                if slot_work is not None and i == 1:
                    slot_work()
            issue_pv(len(pairs) - 1)
            if slot_work is not None and len(pairs) == 1:
                slot_work()

            # evict OT unnormalized (frees otps bank); den + normalize are
            # deferred to finalize_strip so the PE never waits on the acc chain
            ot_sb = otp.tile([128, STRIP], BF16, name=f"otsb{s}_{h}", tag="ot")
            nc.vector.tensor_copy(ot_sb[:], ot_ps[:])
            return ot_sb, acc

        def finalize_strip(s, heads_acc):
            """Softmax denominators + in-place OT normalization for all 4 heads."""
            for h, (ot_sb, acc) in enumerate(heads_acc):
                den_ps = auxps.tile([1, STRIP], F32, name=f"den{s}_{h}", tag="aux")
                nc.tensor.matmul(den_ps[:], ones_sb[:], acc[:, :STRIP],
                                 start=True, stop=False)
                nc.tensor.matmul(den_ps[:], ones_sb[:], acc[:, STRIP:],
                                 start=False, stop=True)
                dr = rowp.tile([1, STRIP], F32, name=f"dr{s}_{h}", tag="rrows")
                nc.vector.reciprocal_approx_fast(dr[:], den_ps[:])
                bcd = bcp.tile([128, STRIP], F32, name=f"dbc{s}_{h}", tag="bc")
                nc.gpsimd.partition_broadcast(bcd[:], dr[:])
                nc.vector.tensor_tensor(ot_sb[:], ot_sb[:], bcd[:], mult)
            return [ot for ot, _ in heads_acc]

        # ================= schedule =================
        # Prologue (DMA-bound): KV3, KV2, Q3, KV1, KV0 — ordered to ride the
        # xt DMA stream (token-hi, wq, token-lo). All Sqrts land pre-Exp.
        project_kv(3)
        project_kv(2)
        qt3 = project_q_inline(3)
        project_kv(1)
        project_kv(0)

        RAW_PLAN = {0: (0,), 1: (1,), 2: (2,), 3: (3,)}

        # attn(3): interleave next strip's Q raw steps (Square only — table-safe)
        q_raws = []
        rows4 = rsbp.tile([1, HQ * STRIP], F32, name="rows4_2", tag="rows4")
        heads_acc = []
        for h in range(HQ):
            def work3(h=h):
                for r in RAW_PLAN[h]:
                    q_raws.append(q_raw_step(2, r, rows4))
            heads_acc.append(attn_head(3, h, qt3, slot_work=work3))
        qt_next = finalize_q(2, q_raws, rows4)
        pending_out.append((3, finalize_strip(3, heads_acc)))

        # strips 2, 1: attention + pending out-proj + next strip's Q raws
        for s in (2, 1):
            qt_s = qt_next
            q_raws = []
            rows4 = rsbp.tile([1, HQ * STRIP], F32, name=f"rows4_{s-1}", tag="rows4")
            heads_acc = []
            for h in range(HQ):
                def work(h=h, s=s, rows4=rows4):
                    ps_, heads_ = pending_out[0]
                    emit_out_chunk(ps_, heads_, h)
                    for r in RAW_PLAN[h]:
                        q_raws.append(q_raw_step(s - 1, r, rows4))
                heads_acc.append(attn_head(s, h, qt_s, slot_work=work))
            pending_out.pop(0)
            qt_next = finalize_q(s - 1, q_raws, rows4)
            pending_out.append((s, finalize_strip(s, heads_acc)))

        # strip 0: attention + pending out-proj; den chains per head so the
        # tail's final out-proj isn't gated on a batched finalize
        qt_s = qt_next
        final_heads = []
        for h in range(HQ):
            def work0(h=h):
                ps_, heads_ = pending_out[0]
                emit_out_chunk(ps_, heads_, h)
            ha = attn_head(0, h, qt_s, slot_work=work0)
            final_heads.append(finalize_strip(0, [ha])[0])
        pending_out.pop(0)

        # final out-proj for strip 0 (otps free: ping-pong banks)
        for tb in range(4):
            emit_out_chunk(0, final_heads, tb, pingpong=True)

    nc.compile()
    return nc


_NC_CACHE = None
last_result = None


def _tri_np():
    kr = np.arange(128)[:, None]
    qc = np.arange(128)[None, :]
    return np.where(kr <= qc, 1.0, 0.0).astype(np.float32)


def _pack_ktiles(a):
    """[KT*128, W] -> [128, KT*W] (k-tiles concatenated along free dim)."""
    kt, rem = divmod(a.shape[0], 128)
    assert rem == 0
    w = a.shape[1]
    return np.ascontiguousarray(
        a.reshape(kt, 128, w).transpose(1, 0, 2).reshape(128, kt * w))


def kernel(x, Wq, Wk, Wv, Wo, gq, gk):
    global _NC_CACHE, last_result
    import ml_dtypes

    bf16 = ml_dtypes.bfloat16
    x = np.asarray(x, np.float32)
    Wq = np.asarray(Wq, np.float32)
    Wk = np.asarray(Wk, np.float32)
    Wv = np.asarray(Wv, np.float32)
    Wo = np.asarray(Wo, np.float32)
    gq = np.asarray(gq, np.float32)
    gk = np.asarray(gk, np.float32)

    tri = _tri_np().astype(bf16)
    ones = np.ones((128, 1), np.float32).astype(bf16)
    ident = np.eye(128, dtype=np.float32).astype(bf16)
    in_maps = []
    for core in range(NCORES):
        b, g = core // 4, core % 4
        in_maps.append({
            "xt": _pack_ktiles(np.ascontiguousarray(x[b].T)).astype(bf16),
            "wq": _pack_ktiles(Wq[:, g * HQ * D:(g + 1) * HQ * D]).astype(bf16),
            "wk": _pack_ktiles(Wk[:, g * D:(g + 1) * D]).astype(bf16),
            "wv": _pack_ktiles(Wv[:, g * D:(g + 1) * D]).astype(bf16),
            "wo": np.ascontiguousarray(Wo[g * HQ * D:(g + 1) * HQ * D, :]).astype(bf16),
            "gq": np.ascontiguousarray(gq.reshape(D, 1)),
            "gk": np.ascontiguousarray(gk.reshape(D, 1)),
            "tri": tri,
            "ones": ones,
            "ident": ident,
        })

    if TRACE:
        _install_profile_shim()
    if _NC_CACHE is None:
        _NC_CACHE = build()
    last_result = run_bass_kernel_spmd(
        _NC_CACHE, in_maps, core_ids=list(range(NCORES)), trace=TRACE
    )
    out = np.zeros((2, S, HID), np.float32)
    for core in range(NCORES):
        out[core // 4] += np.asarray(last_result.results[core]["out"]).astype(np.float32)
    return out


# revision 44
# speedup vs baseline: 1.2071x; 1.1946x over previous
"""Distributed Bass kernel for nn_Attention_75514114998541.

GQA attention block (16 Q heads / 4 KV heads, head_dim 128, hidden 2048,
B=2, S=2048) with per-head RMSNorm on q/k, causal softmax, output proj.

Sharding: 8 cores = 2 (batch) x 4 (head groups). Core 4*b+g handles batch b
and heads [4g, 4g+4) (= kv head g). Wq/Wk/Wv column-sharded, Wo row-sharded;
each core emits a partial [S, HID] output (bf16), host sums partials in f32.

All matmul operands are bf16 (full PE rate at any N, half the DMA bytes);
accumulation stays f32 in PSUM. Feature-on-partition layout throughout:
  xT[hid, tok] -> QT/KT[d, tok] -> ST[k, q] -> PT -> OT[d, q] -> out[tok, hid]

Host packs xt/wq/wk/wv so every DMA moves >=2KB per partition row (1KB rows
halve DMA efficiency). Schedule: K/V projections for all strips first (xt
streamed token-hi then token-lo halves), then q-strips in REVERSE (3,2,1,0)
so the longest causal softmax chain overlaps the input-DMA prologue. Q
projections for strip s-1 are split: matmuls+Square (ACT-table-safe)
interleave into attn(s)'s per-head slots; the Sqrt/recip/bcast/STT tail runs
at the strip boundary so Sqrt never lands between Exps (2 ACT table loads
per strip). Out-proj of strip s is deferred into strip s-1's attention.

ST/exp/acc run on paired [128,1024] PSUM tiles (2 banks) to amortize ACT/DVE
per-instruction overhead. Causal masking: diag tiles get a gpsimd memset of
the dead prefix + 128x128 lower-tri mask multiply (gpsimd).
PSUM: 2x ST-pair (4) + PV accum (1) + aux raws/rows/dens/chunks (3) = 8.
"""
import contextlib
import ctypes
import os
import sys
import types

import numpy as np

sys.path.insert(0, "/opt/trn_rl_repo")

import concourse.bacc as bacc
import concourse.mybir as mybir
import concourse.tile as tile
from concourse.bass_utils import run_bass_kernel_spmd

F32 = mybir.dt.float32
BF16 = mybir.dt.bfloat16

NCORES = 8
S = 2048            # sequence length (= tokens per batch)
HID = 2048          # hidden dim
D = 128             # head dim
HQ = 4              # q heads per core
STRIP = 512         # token strip (matmul moving free dim)
NSTRIP = S // STRIP          # 4
KT = HID // 128              # 16 hidden k-tiles
EPS = 1e-6
TRACE = os.environ.get("BASS_KERNEL_TRACE", "0") == "1"


def _install_profile_shim():
    """antenv.axon_hooks shim so trace=True captures NTFF under axon."""
    if "antenv.axon_hooks" in sys.modules:
        return
    so_path = "/opt/axon/libaxon_pjrt.so"
    try:
        lib = ctypes.CDLL(so_path)
    except OSError:
        return
    if not hasattr(lib, "axon_start_nrt_profile"):
        return
    lib.axon_start_nrt_profile.argtypes = [ctypes.POINTER(ctypes.c_int64), ctypes.c_size_t]
    lib.axon_start_nrt_profile.restype = ctypes.c_int64
    lib.axon_stop_nrt_profile.argtypes = [ctypes.c_char_p]
    lib.axon_stop_nrt_profile.restype = ctypes.c_int64

    @contextlib.contextmanager
    def _hook(output_dir, device_ids):
        import jax

        jax.devices()
        if device_ids:
            ids = (ctypes.c_int64 * len(device_ids))(*device_ids)
            rc = lib.axon_start_nrt_profile(ids, len(device_ids))
        else:
            rc = lib.axon_start_nrt_profile(None, 0)
        if rc != 0:
            raise RuntimeError(f"axon_start_nrt_profile rc={rc}")
        try:
            yield
        finally:
            n = lib.axon_stop_nrt_profile(str(output_dir).encode())
            if n < 0:
                raise RuntimeError(f"axon_stop_nrt_profile rc={n}")

    mod = types.ModuleType("antenv.axon_hooks")
    state = {"hook": _hook}
    mod.set_axon_ntff_profile_hook = lambda h: state.update(hook=h)
    mod.get_axon_ntff_profile_hook = lambda: state["hook"]
    sys.modules["antenv.axon_hooks"] = mod
    try:
        import antenv

        antenv.axon_hooks = mod
    except ImportError:
        pass


def build():
    nc = bacc.Bacc("TRN2", target_bir_lowering=False, debug=False, num_devices=NCORES)

    # host-packed layouts: k-tiles concatenated along the free dim so DMA
    # rows are 2-16KB per partition
    xt_ext = nc.dram_tensor("xt", [128, KT * S], BF16, kind="ExternalInput")
    wq_ext = nc.dram_tensor("wq", [128, KT * HQ * D], BF16, kind="ExternalInput")
    wk_ext = nc.dram_tensor("wk", [128, KT * D], BF16, kind="ExternalInput")
    wv_ext = nc.dram_tensor("wv", [128, KT * D], BF16, kind="ExternalInput")
    wo_ext = nc.dram_tensor("wo", [HQ * D, HID], BF16, kind="ExternalInput")
    gq_ext = nc.dram_tensor("gq", [D, 1], F32, kind="ExternalInput")
    gk_ext = nc.dram_tensor("gk", [D, 1], F32, kind="ExternalInput")
    tri_ext = nc.dram_tensor("tri", [128, 128], BF16, kind="ExternalInput")
    ones_ext = nc.dram_tensor("ones", [128, 1], BF16, kind="ExternalInput")
    ident_ext = nc.dram_tensor("ident", [128, 128], BF16, kind="ExternalInput")
    out_ext = nc.dram_tensor("out", [S, HID], BF16, kind="ExternalOutput")

    Exp = mybir.ActivationFunctionType.Exp
    Sqrt = mybir.ActivationFunctionType.Sqrt
    Square = mybir.ActivationFunctionType.Square
    mult = mybir.AluOpType.mult
    scale_qk = float(D) ** -0.5

    with tile.TileContext(nc) as tc, contextlib.ExitStack() as ctx:
        # ---- SBUF pools
        wpool = ctx.enter_context(tc.tile_pool(name="w", bufs=1))
        cpool = ctx.enter_context(tc.tile_pool(name="c", bufs=1))
        kvp = ctx.enter_context(tc.tile_pool(name="kv", bufs=1))
        qtp = ctx.enter_context(tc.tile_pool(name="qt", bufs=5))
        qrp = ctx.enter_context(tc.tile_pool(name="qr", bufs=5))
        otp = ctx.enter_context(tc.tile_pool(name="ot", bufs=8))
        ptp = ctx.enter_context(tc.tile_pool(name="pt", bufs=4))
        scr = ctx.enter_context(tc.tile_pool(name="scr", bufs=2))
        rowp = ctx.enter_context(tc.tile_pool(name="rows", bufs=2))
        rsbp = ctx.enter_context(tc.tile_pool(name="rsb", bufs=2))
        bcp = ctx.enter_context(tc.tile_pool(name="bc", bufs=4))
        outp = ctx.enter_context(tc.tile_pool(name="outev", bufs=2))
        accp = ctx.enter_context(tc.tile_pool(name="accp", bufs=5))
        # ---- PSUM pools: 4 + 1 + 3 = 8 banks
        stps = ctx.enter_context(tc.tile_pool(name="stps", bufs=2, space="PSUM"))    # [128,1024] ST pairs + vtp
        otps = ctx.enter_context(tc.tile_pool(name="otps", bufs=1, space="PSUM"))    # [128,512] PV accum
        auxps = ctx.enter_context(tc.tile_pool(name="auxps", bufs=3, space="PSUM"))  # raws/rows/dens/chunks

        # ---- DMA program order = arrival order.
        gq_sb = cpool.tile([D, 1], F32, name="gq_sb", tag="gq_sb")
        nc.sync.dma_start(out=gq_sb[:], in_=gq_ext[:])
        gk_sb = cpool.tile([D, 1], F32, name="gk_sb", tag="gk_sb")
        nc.sync.dma_start(out=gk_sb[:], in_=gk_ext[:])
        tri_sb = cpool.tile([128, 128], BF16, name="tri_sb", tag="tri_sb")
        nc.sync.dma_start(out=tri_sb[:], in_=tri_ext[:])
        ones_sb = cpool.tile([128, 1], BF16, name="ones_sb", tag="ones_sb")
        nc.sync.dma_start(out=ones_sb[:], in_=ones_ext[:])
        ident_sb = cpool.tile([128, 128], BF16, name="ident_sb", tag="ident_sb")
        nc.sync.dma_start(out=ident_sb[:], in_=ident_ext[:])
        eps_sb = cpool.tile([1, 1], F32, name="eps_sb", tag="eps_sb")
        nc.vector.memset(eps_sb[:], EPS)

        wk_sb = wpool.tile([128, KT * D], BF16, name="wk_sb", tag="wk")
        nc.sync.dma_start(out=wk_sb[:], in_=wk_ext[:])
        xt_sb = wpool.tile([128, KT * S], BF16, name="xt_sb", tag="xt")
        # first hi-chunks early so KV3's first matmuls start ASAP, then wv,
        # then the rest of token-hi (strips 3,2), then wq, then token-lo
        for k in range(2):
            nc.sync.dma_start(out=xt_sb[:, k * S + S // 2:(k + 1) * S],
                              in_=xt_ext[:, k * S + S // 2:(k + 1) * S])
        wv_sb = wpool.tile([128, KT * D], BF16, name="wv_sb", tag="wv")
        nc.sync.dma_start(out=wv_sb[:], in_=wv_ext[:])
        for k in range(2, KT):
            nc.sync.dma_start(out=xt_sb[:, k * S + S // 2:(k + 1) * S],
                              in_=xt_ext[:, k * S + S // 2:(k + 1) * S])
        wq_sb = wpool.tile([128, KT * HQ * D], BF16, name="wq_sb", tag="wq")
        for j in range(4):
            w = KT * HQ * D // 4
            nc.sync.dma_start(out=wq_sb[:, j * w:(j + 1) * w],
                              in_=wq_ext[:, j * w:(j + 1) * w])
        for k in range(KT):
            nc.sync.dma_start(out=xt_sb[:, k * S:k * S + S // 2],
                              in_=xt_ext[:, k * S:k * S + S // 2])
        wo_t = []
        for h in range(HQ):
            wo_h = wpool.tile([128, HID], BF16, name=f"wo{h}", tag=f"wo{h}")
            nc.sync.dma_start(out=wo_h[:], in_=wo_ext[h * 128:(h + 1) * 128, :])
            wo_t.append(wo_h)

        def xt_ap(s, k):
            return xt_sb[:, k * S + s * STRIP:k * S + (s + 1) * STRIP]

        def wq_ap(k, h):
            return wq_sb[:, k * HQ * D + h * D:k * HQ * D + (h + 1) * D]

        kt_strips = [None] * NSTRIP   # K-hat-T strips [128 d, STRIP tok] bf16
        v_strips = [None] * NSTRIP    # V strips [128 tok-blk, 4*128 d] bf16
        pending_out = []              # deferred out-proj: (strip, [4 x ot_sb])

        def row_invrms_bcast(row_sb, suffix, scale):
            """row_sb [1,512] sumsq (SBUF or PSUM) -> [128,512] f32 bcast of 1/rms."""
            rms = rowp.tile([1, STRIP], F32, name=f"rms_{suffix}", tag="rows")
            nc.scalar.activation(rms[:], row_sb[:], Sqrt,
                                 bias=eps_sb[:], scale=scale)
            rr = rowp.tile([1, STRIP], F32, name=f"rr_{suffix}", tag="rrows")
            nc.vector.reciprocal_approx_fast(rr[:], rms[:])
            bc = bcp.tile([128, STRIP], F32, name=f"bc_{suffix}", tag="bc")
            nc.gpsimd.partition_broadcast(bc[:], rr[:])
            return bc

        def project_q_inline(s):
            """Full Q projection + rmsnorm for strip s (prologue only)."""
            qt_h = []
            for h in range(HQ):
                raw = auxps.tile([128, STRIP], F32, name=f"qraw{s}_{h}", tag="aux")
                for k in range(KT):
                    nc.tensor.matmul(raw[:], wq_ap(k, h), xt_ap(s, k),
                                     start=(k == 0), stop=(k == KT - 1))
                sq = scr.tile([128, STRIP], BF16, name=f"sq{s}_{h}", tag="sq")
                nc.scalar.activation(sq[:], raw[:], Square)
                row_ps = auxps.tile([1, STRIP], F32, name=f"qrow{s}_{h}", tag="aux")
                nc.tensor.matmul(row_ps[:], ones_sb[:], sq[:], start=True, stop=True)
                bc = row_invrms_bcast(row_ps, f"q{s}_{h}", 1.0 / D)
                qn = qtp.tile([128, STRIP], BF16, name=f"qt{s}_{h}", tag="qt")
                nc.vector.scalar_tensor_tensor(qn[:], raw[:], gq_sb[:], bc[:],
                                               mult, mult)
                qt_h.append(qn)
            return qt_h

        def q_raw_step(s, h, rows4):
            """Q matmuls + Square + rowsum for (s,h); norm tail deferred.

            Square is in the same ACT table set as Exp, so this is safe to
            interleave into an attention phase. The sumsq row lands in slice
            h of rows4 so ONE batched Sqrt per strip handles all heads (the
            Sqrt table-set swap happens once, after h3's row is ready).
            """
            raw = auxps.tile([128, STRIP], F32, name=f"qraw{s}_{h}", tag="aux")
            for k in range(KT):
                nc.tensor.matmul(raw[:], wq_ap(k, h), xt_ap(s, k),
                                 start=(k == 0), stop=(k == KT - 1))
            sq = scr.tile([128, STRIP], BF16, name=f"sq{s}_{h}", tag="sq")
            nc.scalar.activation(sq[:], raw[:], Square)
            qraw_sb = qrp.tile([128, STRIP], BF16, name=f"qrsb{s}_{h}", tag="qr")
            nc.vector.tensor_copy(qraw_sb[:], raw[:])
            row_ps = auxps.tile([1, STRIP], F32, name=f"qrow{s}_{h}", tag="aux")
            nc.tensor.matmul(row_ps[:], ones_sb[:], sq[:], start=True, stop=True)
            nc.vector.tensor_copy(rows4[0:1, h * STRIP:(h + 1) * STRIP], row_ps[:])
            return qraw_sb

        def finalize_q(s, raws, rows4):
            """Deferred Q norm tail: one batched Sqrt, then per-head
            recip/bcast/STT (prompt chains for attn(s) head 0)."""
            rms4 = rsbp.tile([1, HQ * STRIP], F32, name=f"rms4_{s}", tag="rms4", bufs=1)
            nc.scalar.activation(rms4[:], rows4[:], Sqrt,
                                 bias=eps_sb[:], scale=1.0 / D)
            qt_h = []
            for h, qraw_sb in enumerate(raws):
                rr = rowp.tile([1, STRIP], F32, name=f"rr_q{s}_{h}", tag="rrows")
                nc.vector.reciprocal_approx_fast(
                    rr[:], rms4[0:1, h * STRIP:(h + 1) * STRIP])
                bc = bcp.tile([128, STRIP], F32, name=f"bc_q{s}_{h}", tag="bc")
                nc.gpsimd.partition_broadcast(bc[:], rr[:])
                qn = qtp.tile([128, STRIP], BF16, name=f"qt{s}_{h}", tag="qt")
                nc.vector.scalar_tensor_tensor(qn[:], qraw_sb[:], gq_sb[:], bc[:],
                                               mult, mult)
                qt_h.append(qn)
            return qt_h

        def project_kv(s):
            """K/V projection for strip s (prologue; Sqrt is pre-exp here)."""
            kraw = auxps.tile([128, STRIP], F32, name=f"kraw{s}", tag="aux")
            for k in range(KT):
                nc.tensor.matmul(kraw[:], wk_sb[:, k * D:(k + 1) * D], xt_ap(s, k),
                                 start=(k == 0), stop=(k == KT - 1))
            sqk = scr.tile([128, STRIP], BF16, name=f"sqk{s}", tag="sq")
            nc.scalar.activation(sqk[:], kraw[:], Square)
            vraw = auxps.tile([128, STRIP], F32, name=f"vraw{s}", tag="aux")
            for k in range(KT):
                nc.tensor.matmul(vraw[:], wv_sb[:, k * D:(k + 1) * D], xt_ap(s, k),
                                 start=(k == 0), stop=(k == KT - 1))
            krow_ps = auxps.tile([1, STRIP], F32, name=f"krow{s}", tag="aux")
            nc.tensor.matmul(krow_ps[:], ones_sb[:], sqk[:], start=True, stop=True)
            bck = row_invrms_bcast(krow_ps, f"k{s}", 1.0 / D)
            kn = kvp.tile([128, STRIP], BF16, name=f"kt_strip{s}", tag=f"kt{s}")
            nc.vector.scalar_tensor_tensor(kn[:], kraw[:], gk_sb[:], bck[:], mult, mult)
            kt_strips[s] = kn
            # V: evict [d, tok] to SBUF, then transpose via stps slots
            vt_sb = scr.tile([128, STRIP], BF16, name=f"vt_sb{s}", tag=f"vtsb{s}", bufs=1)
            nc.vector.tensor_copy(vt_sb[:], vraw[:])
            vs = kvp.tile([128, 4 * D], BF16, name=f"v{s}", tag=f"v{s}")
            for tb in range(4):
                tp = stps.tile([128, D], BF16, name=f"vtp{s}_{tb}", tag="st")
                nc.tensor.transpose(tp[:], vt_sb[:, tb * 128:(tb + 1) * 128],
                                    ident_sb[:])
                nc.vector.tensor_copy(vs[:, tb * D:(tb + 1) * D], tp[:])
            v_strips[s] = vs

        def v_tile(k):
            return v_strips[k // 4][:, (k % 4) * D:((k % 4) + 1) * D]

        def kt_block(k):
            return kt_strips[k // 4][:, (k % 4) * 128:((k % 4) + 1) * 128]

        def emit_out_chunk(ps_, heads_, tb, pingpong=False):
            """Emit the 4 hs-chunks of token-block tb for pending strip ps_.

            Pairs of chunks share one [128,1024] bf16 ob tile -> one 2KB-row
            DMA per pair. pingpong alternates aux/ot PSUM banks (only legal
            when no PV accumulation is live — final strip).
            """
            tok0 = ps_ * STRIP + tb * 128
            for hp in range(2):
                ob = outp.tile([128, 2 * STRIP], BF16, name=f"ob{ps_}_{tb}_{hp}", tag="ob")
                for hh in range(2):
                    hs = 2 * hp + hh
                    if pingpong and hs % 2 == 1:
                        op_ps = otps.tile([128, STRIP], F32, name=f"op{ps_}_{tb}_{hs}", tag="ot")
                    else:
                        op_ps = auxps.tile([128, STRIP], F32, name=f"op{ps_}_{tb}_{hs}", tag="aux")
                    for h in range(HQ):
                        nc.tensor.matmul(
                            op_ps[:],
                            heads_[h][:, tb * 128:(tb + 1) * 128],
                            wo_t[h][:, hs * STRIP:(hs + 1) * STRIP],
                            start=(h == 0), stop=(h == HQ - 1),
                        )
                    if hh == 0 or pingpong:
                        nc.scalar.copy(ob[:, hh * STRIP:(hh + 1) * STRIP], op_ps[:])
                    else:
                        nc.vector.tensor_copy(ob[:, STRIP:], op_ps[:])
                nc.sync.dma_start(
                    out=out_ext[tok0:tok0 + 128, hp * 2 * STRIP:(hp + 1) * 2 * STRIP],
                    in_=ob[:],
                )

        def attn_head(s, h, qt_h, slot_work=None):
            """Attention for (strip s, head h). k-pairs diag-first then descending.

            slot_work: optional callback issued mid-stream (after the 2nd
            pair) — used for out-proj chunks and next-strip Q raw steps.
            """
            ot_ps = otps.tile([128, STRIP], F32, name=f"ot{s}_{h}", tag="ot")
            acc = accp.tile([128, 2 * STRIP], BF16, name=f"acc{s}_{h}", tag="acc")

            pairs = [(4 * s + 1, 4 * s + 0, True), (4 * s + 3, 4 * s + 2, True)]
            ks = list(range(4 * s - 1, -1, -1))
            for i in range(0, len(ks), 2):
                pairs.append((ks[i], ks[i + 1], False))

            pts = {}

            def issue_st(idx):
                k0, k1, diag = pairs[idx]
                st = stps.tile([128, 2 * STRIP], F32, name=f"st{s}_{h}_{idx}", tag="st")
                for half, k in enumerate((k0, k1)):
                    off = half * STRIP
                    j = k - 4 * s
                    m0 = 128 * j if j > 0 else 0
                    nc.tensor.matmul(
                        st[:, off + m0:off + STRIP],
                        kt_block(k), qt_h[h][:, m0:],
                        start=True, stop=True,
                    )
                pt = ptp.tile([128, 2 * STRIP], BF16, name=f"pt{s}_{h}_{idx}", tag="pt")
                if diag:
                    for half, k in enumerate((k0, k1)):
                        off = half * STRIP
                        j = k - 4 * s
                        c0 = 128 * j
                        if c0 > 0:
                            nc.gpsimd.memset(pt[:, off:off + c0], 0.0)
                        nc.scalar.activation(pt[:, off + c0:off + STRIP],
                                             st[:, off + c0:off + STRIP],
                                             Exp, scale=scale_qk)
                        nc.vector.tensor_tensor(pt[:, off + c0:off + c0 + 128],
                                                pt[:, off + c0:off + c0 + 128],
                                                tri_sb[:], mult)
                else:
                    nc.scalar.activation(pt[:], st[:], Exp, scale=scale_qk)
                pts[idx] = pt

            def issue_pv(idx):
                k0, k1, diag = pairs[idx]
                first = idx == 0
                last = idx == len(pairs) - 1
                for half, k in enumerate((k0, k1)):
                    off = half * STRIP
                    j = k - 4 * s
                    m0 = 128 * j if j > 0 else 0
                    nc.tensor.matmul(
                        ot_ps[:, m0:], v_tile(k), pts[idx][:, off + m0:off + STRIP],
                        start=(first and half == 0),
                        stop=(last and half == 1),
                    )
                if first:
                    nc.vector.tensor_copy(acc[:], pts[idx][:])
                else:
                    nc.vector.tensor_add(acc[:], acc[:], pts[idx][:])
                del pts[idx]

            issue_st(0)
            for i in range(1, len(pairs)):
                issue_st(i)
                issue_pv(i - 1)
                if slot_work is not None and i == 1:
                    slot_work()
            issue_pv(len(pairs) - 1)
            if slot_work is not None and len(pairs) == 1:
                slot_work()

            # evict OT unnormalized (frees otps bank); den + normalize are
            # deferred to finalize_strip so the PE never waits on the acc chain
            ot_sb = otp.tile([128, STRIP], BF16, name=f"otsb{s}_{h}", tag="ot")
            nc.vector.tensor_copy(ot_sb[:], ot_ps[:])
            return ot_sb, acc

        def finalize_strip(s, heads_acc):
            """Softmax denominators + in-place OT normalization for all 4 heads."""
            for h, (ot_sb, acc) in enumerate(heads_acc):
                den_ps = auxps.tile([1, STRIP], F32, name=f"den{s}_{h}", tag="aux")
                nc.tensor.matmul(den_ps[:], ones_sb[:], acc[:, :STRIP],
                                 start=True, stop=False)
                nc.tensor.matmul(den_ps[:], ones_sb[:], acc[:, STRIP:],
                                 start=False, stop=True)
                dr = rowp.tile([1, STRIP], F32, name=f"dr{s}_{h}", tag="rrows")
                nc.vector.reciprocal_approx_fast(dr[:], den_ps[:])
                bcd = bcp.tile([128, STRIP], F32, name=f"dbc{s}_{h}", tag="bc")
                nc.gpsimd.partition_broadcast(bcd[:], dr[:])
                nc.vector.tensor_tensor(ot_sb[:], ot_sb[:], bcd[:], mult)
            return [ot for ot, _ in heads_acc]

        # ================= schedule =================
        # Prologue (DMA-bound): KV3, KV2, Q3, KV1, KV0 — ordered to ride the
        # xt DMA stream (token-hi, wq, token-lo). All Sqrts land pre-Exp.
        project_kv(3)
        project_kv(2)
        qt3 = project_q_inline(3)
        project_kv(1)
        project_kv(0)

        # h2 carries two raw steps so the batched Sqrt (and its ACT table
        # swap) fires a head-slot before the strip boundary
        RAW_PLAN = {0: (0,), 1: (1,), 2: (2, 3), 3: ()}

        # attn(3): interleave next strip's Q raw steps (Square only — table-safe)
        q_raws = []
        rows4 = rsbp.tile([1, HQ * STRIP], F32, name="rows4_2", tag="rows4")
        heads_acc = []
        for h in range(HQ):
            def work3(h=h):
                for r in RAW_PLAN[h]:
                    q_raws.append(q_raw_step(2, r, rows4))
            heads_acc.append(attn_head(3, h, qt3, slot_work=work3))
        qt_next = finalize_q(2, q_raws, rows4)
        pending_out.append((3, finalize_strip(3, heads_acc)))

        # strips 2, 1: attention + pending out-proj + next strip's Q raws
        for s in (2, 1):
            qt_s = qt_next
            q_raws = []
            rows4 = rsbp.tile([1, HQ * STRIP], F32, name=f"rows4_{s-1}", tag="rows4")
            heads_acc = []
            for h in range(HQ):
                def work(h=h, s=s, rows4=rows4):
                    ps_, heads_ = pending_out[0]
                    emit_out_chunk(ps_, heads_, h)
                    for r in RAW_PLAN[h]:
                        q_raws.append(q_raw_step(s - 1, r, rows4))
                heads_acc.append(attn_head(s, h, qt_s, slot_work=work))
            pending_out.pop(0)
            qt_next = finalize_q(s - 1, q_raws, rows4)
            pending_out.append((s, finalize_strip(s, heads_acc)))

        # strip 0: attention + pending out-proj; den chains per head so the
        # tail's final out-proj isn't gated on a batched finalize
        qt_s = qt_next
        final_heads = []
        for h in range(HQ):
            def work0(h=h):
                ps_, heads_ = pending_out[0]
                emit_out_chunk(ps_, heads_, h)
            ha = attn_head(0, h, qt_s, slot_work=work0)
            final_heads.append(finalize_strip(0, [ha])[0])
        pending_out.pop(0)

        # final out-proj for strip 0 (otps free: ping-pong banks)
        for tb in range(4):
            emit_out_chunk(0, final_heads, tb, pingpong=True)

    nc.compile()
    return nc


_NC_CACHE = None
last_result = None


def _tri_np():
    kr = np.arange(128)[:, None]
    qc = np.arange(128)[None, :]
    return np.where(kr <= qc, 1.0, 0.0).astype(np.float32)


def _pack_ktiles(a):
    """[KT*128, W] -> [128, KT*W] (k-tiles concatenated along free dim)."""
    kt, rem = divmod(a.shape[0], 128)
    assert rem == 0
    w = a.shape[1]
    return np.ascontiguousarray(
        a.reshape(kt, 128, w).transpose(1, 0, 2).reshape(128, kt * w))


def kernel(x, Wq, Wk, Wv, Wo, gq, gk):
    global _NC_CACHE, last_result
    import ml_dtypes

    bf16 = ml_dtypes.bfloat16
    x = np.asarray(x, np.float32)
    Wq = np.asarray(Wq, np.float32)
    Wk = np.asarray(Wk, np.float32)
    Wv = np.asarray(Wv, np.float32)
    Wo = np.asarray(Wo, np.float32)
    gq = np.asarray(gq, np.float32)
    gk = np.asarray(gk, np.float32)

    tri = _tri_np().astype(bf16)
    ones = np.ones((128, 1), np.float32).astype(bf16)
    ident = np.eye(128, dtype=np.float32).astype(bf16)
    in_maps = []
    for core in range(NCORES):
        b, g = core // 4, core % 4
        in_maps.append({
            "xt": _pack_ktiles(np.ascontiguousarray(x[b].T)).astype(bf16),
            "wq": _pack_ktiles(Wq[:, g * HQ * D:(g + 1) * HQ * D]).astype(bf16),
            "wk": _pack_ktiles(Wk[:, g * D:(g + 1) * D]).astype(bf16),
            "wv": _pack_ktiles(Wv[:, g * D:(g + 1) * D]).astype(bf16),
            "wo": np.ascontiguousarray(Wo[g * HQ * D:(g + 1) * HQ * D, :]).astype(bf16),
            "gq": np.ascontiguousarray(gq.reshape(D, 1)),
            "gk": np.ascontiguousarray(gk.reshape(D, 1)),
            "tri": tri,
            "ones": ones,
            "ident": ident,
        })

    if TRACE:
        _install_profile_shim()
    if _NC_CACHE is None:
        _NC_CACHE = build()
    last_result = run_bass_kernel_spmd(
        _NC_CACHE, in_maps, core_ids=list(range(NCORES)), trace=TRACE
    )
    out = np.zeros((2, S, HID), np.float32)
    for core in range(NCORES):
        out[core // 4] += np.asarray(last_result.results[core]["out"]).astype(np.float32)
    return out
